# revision 5
# baseline (speedup 1.0000x reference)
"""Multi-head attention (B=4, G=2048, C=1024, H=16) on 8 TRN2 NeuronCores.

Sharding: (batch x head-half). Core c handles batch c//2 and an 8-head
slice (c%2): its heads' q/k/v projections, full softmax attention, and a
partial output projection over its 512 channels; the host sums core
pairs and adds the bias.

Single fused pipeline per core (vs. 3 serial phases):
  - x/W operands in bf16 (half the DMA, same 1 col/cycle PE rate);
    qT/kT in f32r for score precision; ex/v/oT/wp in bf16.
  - QKV-projection and V matmuls are interleaved as filler into the
    attention groups' PE stream so the Tensor engine never idles while
    the ACT engine streams the 33.5M exps (the critical co-resource).
  - Scores computed transposed ([k, q]); denominator via ones-column in
    v; exp fused with the 1/sqrt(d) scale on ACT.
  - Normalization: reciprocal straight off the psum ones-row, z-split
    into two pipelined DRAM-round-trip broadcasts + DVE multiplies.
"""

from contextlib import ExitStack

import numpy as np
import ml_dtypes

import concourse.bass as bass
import concourse.tile as tile
from concourse import mybir
from concourse.bass_utils import run_bass_kernel_spmd
from concourse.vector_clock import ScopedClock, VectorClock
from concourse.tile_sem_assignment import N_PROCS

F32 = mybir.dt.float32
F32R = mybir.dt.float32r
BF16 = mybir.dt.bfloat16
NP_BF16 = ml_dtypes.bfloat16

B, G, C, H = 4, 2048, 1024, 16
N_CORES = 8
H_LOC = H // 2          # 8 heads per core
O_LOC = H_LOC * 64      # 512 output channels per core
D = 64
CC = C // 128           # 8 contraction tiles
GC = G // 128           # 16 g-tiles
KC = G // 128           # 16 k-tiles
NT = 4                  # head-tiles (2 heads each)


class SplitDrainTileContext(tile.TileContext):
    """Tail drain limited to one sync wait per instruction.

    This environment's walrus rejects >1 sync wait per instruction, so
    wait on each outstanding proc tick with its own NOP first and emit
    the drain bare.
    """

    def _drain_and_barrier(self, tick_clock, wait_clock):
        g = tick_clock.global_clock
        for p in range(N_PROCS):
            if g[p] > 0:
                nop = self.nc.sync.nop(nofuse=True)
                partial = VectorClock([g[q] if q == p else 0 for q in range(N_PROCS)])
                wait_clock.add_sem_waits(nop.ins, ScopedClock({None: partial}))
        self.nc.sync.drain()
        self.nc.all_engine_barrier()
        assert self.sems is not None
        popped = self.nc._tile_sem_poison_stack.pop()
        assert popped is self._sem_poison
        self.nc.clear_and_free_semaphores(list(self.sems.allocated().values()))
        self.nc.all_engine_barrier()


def split_multi_waits(nc):
    """Hoist extra sync waits onto NOPs before each offending instruction
    (this walrus accepts at most one sync wait per instruction)."""
    n_split = 0
    for f in nc.m.functions:
        for bb in f.blocks:
            insts = bb.instructions
            out = []
            for inst in insts:
                si = inst.sync_info
                waits = list(si.on_wait) if si and si.on_wait else []
                if len(waits) > 1:
                    for w in waits[:-1]:
                        nop = mybir.InstNoOp(
                            name=f"{inst.name}_w{n_split}",
                            engine=inst.engine,
                            ins=[],
                            outs=[],
                            sync_info=mybir.SyncInfo(on_wait=[w], on_update=[]),
                        )
                        out.append(nop)
                        n_split += 1
                    inst.sync_info = mybir.SyncInfo(
                        on_wait=[waits[-1]],
                        on_update=list(si.on_update) if si.on_update else [],
                    )
                out.append(inst)
            if len(out) != len(insts):
                bb.instructions[:] = out
    return n_split


def build_program():
    scale = D ** -0.5

    nc = bass.Bass()
    xT = nc.declare_dram_parameter("xT", [C, G], BF16, isOutput=False)
    wqT = nc.declare_dram_parameter("wqT", [C, O_LOC], BF16, isOutput=False)
    wkT = nc.declare_dram_parameter("wkT", [C, O_LOC], BF16, isOutput=False)
    wvT = nc.declare_dram_parameter("wvT", [C, O_LOC], BF16, isOutput=False)
    wpT = nc.declare_dram_parameter("wpT", [O_LOC, C], BF16, isOutput=False)
    out_p = nc.declare_dram_parameter("out_p", [G, C], BF16, isOutput=True)
    out_q = nc.declare_dram_parameter("out_q", [G, C], BF16, isOutput=True)

    rcp_dram = nc.dram_tensor("rcp_scratch", [H_LOC * 2, 1024], F32)

    with SplitDrainTileContext(nc) as tc, ExitStack() as ctx:
        persist = ctx.enter_context(tc.tile_pool(name="persist", bufs=1))
        qT = [persist.tile([128, G], BF16, name=f"qT{t}", tag=f"qT{t}") for t in range(NT)]
        kT = [persist.tile([128, G], BF16, name=f"kT{t}", tag=f"kT{t}") for t in range(NT)]
        oT = [persist.tile([128, G], BF16, name=f"oT{t}", tag=f"oT{t}") for t in range(NT)]
        v_sb = persist.tile([128, GC, H_LOC, 65], BF16, name="v_sb", tag="v_sb")
        wq_sb = persist.tile([128, CC, O_LOC], BF16, name="wq_sb", tag="wq")
        wk_sb = persist.tile([128, CC, O_LOC], BF16, name="wk_sb", tag="wk")
        wv_sb = persist.tile([128, CC, O_LOC], BF16, name="wv_sb", tag="wv")
        wp_sb = persist.tile([128, O_LOC // 128, C], BF16, name="wp_sb", tag="wp")
        xh = [persist.tile([128, CC, 1024], BF16, name=f"xh{g}", tag=f"xh{g}")
              for g in range(2)]

        # Input DMAs, ordered so the prologue's dependencies land first.
        # First K(t0, gh0, c2=0) matmul needs only wk columns 0:128 and
        # x g 0:512 — land those two small slices first (~0.75MB).
        nc.sync.dma_start(
            out=wk_sb[:, :, 0:128],
            in_=wkT.rearrange("(cc p) o -> p cc o", p=128)[:, :, 0:128])
        nc.sync.dma_start(
            out=xh[0][:, :, 0:512],
            in_=xT[:, 0:512].rearrange("(cc p) g -> p cc g", p=128))
        nc.sync.dma_start(
            out=wq_sb[:, :, 0:128],
            in_=wqT.rearrange("(cc p) o -> p cc o", p=128)[:, :, 0:128])
        nc.sync.dma_start(
            out=xh[0][:, :, 512:1024],
            in_=xT[:, 512:1024].rearrange("(cc p) g -> p cc g", p=128))
        nc.sync.dma_start(out=wv_sb[:], in_=wvT.rearrange("(cc p) o -> p cc o", p=128))
        nc.sync.dma_start(
            out=xh[1][:, :, 0:512],
            in_=xT[:, 1024:1536].rearrange("(cc p) g -> p cc g", p=128))
        nc.sync.dma_start(
            out=xh[1][:, :, 512:1024],
            in_=xT[:, 1536:2048].rearrange("(cc p) g -> p cc g", p=128))
        nc.sync.dma_start(
            out=wk_sb[:, :, 128:512],
            in_=wkT.rearrange("(cc p) o -> p cc o", p=128)[:, :, 128:512])
        nc.sync.dma_start(
            out=wq_sb[:, :, 128:512],
            in_=wqT.rearrange("(cc p) o -> p cc o", p=128)[:, :, 128:512])
        nc.sync.dma_start(out=wp_sb[:], in_=wpT.rearrange("(ct p) o -> p ct o", p=128))
        popart = persist.tile([128, GC, C], BF16, name="popart", tag="popart")
        bconst = persist.tile([128, 1024], F32, name="bconst", tag="bconst")
        nc.vector.memset(v_sb[:, :, :, 64:65], 1.0)
        nc.vector.memset(bconst[:], 1064866805.0)

        with tc.tile_pool(name="ex", bufs=8) as expool, \
             tc.tile_pool(name="st", bufs=3) as stpool, \
             tc.tile_pool(name="ex32", bufs=1) as ex32pool, \
             tc.tile_pool(name="den", bufs=1) as dpool, \
             tc.tile_pool(name="bc", bufs=2) as bcpool, \
             tc.tile_pool(name="sc", bufs=2, space="PSUM") as scps, \
             tc.tile_pool(name="av", bufs=1, space="PSUM") as avps, \
             tc.tile_pool(name="pj", bufs=2, space="PSUM") as pjps:

            def emit_proj(dst, w_sb, t, gh, c2):
                # One 512-g chunk of the q or k projection for head-tile t.
                g0 = gh * 1024 + c2 * 512
                ps = pjps.tile([128, 512], F32, name="pj", tag="pj")
                for cc in range(CC):
                    nc.tensor.matmul(
                        ps[:],
                        w_sb[:, cc, t * 128:(t + 1) * 128],
                        xh[gh][:, cc, c2 * 512:(c2 + 1) * 512],
                        start=(cc == 0), stop=(cc == CC - 1),
                    )
                nc.vector.tensor_copy(out=dst[t][:, g0:g0 + 512], in_=ps[:])

            def proj_halves(dst, w_sb, t, gh, c2):
                # emit_proj split into two 4-matmul halves so filler can be
                # spread across the ACT-paced front slots of a group
                g0 = gh * 1024 + c2 * 512
                state = {}
                def mms(ps, lo, hi):
                    for cc in range(lo, hi):
                        nc.tensor.matmul(
                            ps[:],
                            w_sb[:, cc, t * 128:(t + 1) * 128],
                            xh[gh][:, cc, c2 * 512:(c2 + 1) * 512],
                            start=(cc == 0), stop=(cc == CC - 1),
                        )
                def h1():
                    state["ps"] = pjps.tile([128, 512], F32, name="pj", tag="pj")
                    mms(state["ps"], 0, 4)
                def h2():
                    mms(state["ps"], 4, 8)
                    nc.vector.tensor_copy(out=dst[t][:, g0:g0 + 512],
                                          in_=state["ps"][:])
                return h1, h2

            def emit_v(gc):
                # v for one 128-g tile, all 8 heads (512 channels).
                gh, gi = gc // 8, gc % 8
                ps = pjps.tile([128, 512], F32, name="pj", tag="pj")
                for cc in range(CC):
                    nc.tensor.matmul(
                        ps[:],
                        xh[gh][:, cc, gi * 128:(gi + 1) * 128],
                        wv_sb[:, cc, :],
                        start=(cc == 0), stop=(cc == CC - 1),
                    )
                nc.vector.tensor_copy(out=v_sb[:, gc, :, 0:64], in_=ps[:])

            # Filler units: emitted inside attention groups to keep PE fed.
            # Deadlines: kT(t0) second half before (h0,qh0) kc8; V(k) before
            # AV(h0,qh0,k) (lag 5); qT(t0) gh1 before group (h0,qh1);
            # q/k(t1) before h2 (group 4), t2 before h4 (group 8),
            # t3 before h6 (group 12).
            fillers = {}  # (group, slot) -> list of emit fns
            def add(g, s, fn):
                fillers.setdefault((g, s), []).append(fn)

            # group 0 fillers: V0..V15 at slots 0..15, K(t0,gh1) at 2,3
            for k in range(16):
                add(0, k, lambda k=k: emit_v(k))
            add(0, 2, lambda: emit_proj(kT, wk_sb, 0, 1, 0))
            add(0, 3, lambda: emit_proj(kT, wk_sb, 0, 1, 1))
            add(0, 14, lambda: emit_proj(qT, wq_sb, 0, 1, 0))
            add(0, 15, lambda: emit_proj(qT, wq_sb, 0, 1, 1))
            def emit_popart(gc, z):
                # partial output projection over head-tiles 0-2; shipped as
                # out_q and summed on the host. Fills groups 12-15.
                ps = pjps.tile([128, 512], F32, name="pj", tag="pj")
                for ct in range(3):
                    nc.tensor.matmul(
                        ps[:],
                        oT[ct][:, gc * 128:(gc + 1) * 128],
                        wp_sb[:, ct, z * 512:(z + 1) * 512],
                        start=(ct == 0), stop=(ct == 2),
                    )
                nc.vector.tensor_copy(
                    out=popart[:, gc, z * 512:(z + 1) * 512], in_=ps[:])
                if z == 1:
                    nc.sync.dma_start(
                        out=out_q[gc * 128:(gc + 1) * 128, :], in_=popart[:, gc, :])

            def emit_po3(gc):
                # final output projection (head-tile 3) for one g-tile.
                st = stpool.tile([128, C], BF16, name="st", tag="st")
                for z in range(2):
                    ps = pjps.tile([128, 512], F32, name="pj", tag="pj")
                    nc.tensor.matmul(
                        ps[:],
                        oT[3][:, gc * 128:(gc + 1) * 128],
                        wp_sb[:, 3, z * 512:(z + 1) * 512],
                        start=True, stop=True,
                    )
                    if z == 0:
                        nc.scalar.copy(out=st[:, 0:512], in_=ps[:])
                    else:
                        nc.vector.tensor_copy(out=st[:, 512:1024], in_=ps[:])
                nc.sync.dma_start(out=out_p[gc * 128:(gc + 1) * 128, :], in_=st[:])

            # spread t1 over groups 1-3, t2 over 4-7, t3 over 8-11
            # (deadlines: t1 before group 4, t2 before 8, t3 before 12)
            sched = [(1, 0, 1), (2, 0, 1), (2, 8, 1), (3, 0, 1),
                     (4, 0, 2), (5, 0, 2), (6, 0, 2), (7, 0, 2),
                     (8, 0, 3), (9, 0, 3), (10, 0, 3), (11, 0, 3)]
            for i in range(32):
                gc, z = i // 2, i % 2
                add(12 + i // 8, 1 + (i % 8),
                    lambda gc=gc, z=z: emit_popart(gc, z))
            # each entry covers K then Q for (gh, c2) pairs in order
            pair_iter = {1: iter([(0, 0), (0, 1), (1, 0), (1, 1)]),
                         2: iter([(0, 0), (0, 1), (1, 0), (1, 1)]),
                         3: iter([(0, 0), (0, 1), (1, 0), (1, 1)])}
            for (g, s, t) in sched:
                gh, c2 = next(pair_iter[t])
                base = 2 if s == 0 else 7
                k1, k2 = proj_halves(kT, wk_sb, t, gh, c2)
                q1, q2 = proj_halves(qT, wq_sb, t, gh, c2)
                add(g, base, k1)
                add(g, base + 1, k2)
                add(g, base + 2, q1)
                add(g, base + 3, q2)

            # Prologue: k/q(t0) first half, ordered by DMA arrival.
            emit_proj(kT, wk_sb, 0, 0, 0)
            emit_proj(qT, wq_sb, 0, 0, 0)
            emit_proj(kT, wk_sb, 0, 0, 1)
            emit_proj(qT, wq_sb, 0, 0, 1)

            AV_LAG = 7

            for gi in range(16):            # groups: (h, qh)
                h, qh = gi // 2, gi % 2
                t, base = h // 2, (h % 2) * 64
                q0 = qh * 1024
                av = avps.tile([65, 1024], F32, name="av", tag="av")
                ex_t = [None] * KC

                def emit_av(kc):
                    for z in range(2):
                        nc.tensor.matmul(
                            av[:, z * 512:(z + 1) * 512],
                            v_sb[:, kc, h, :],
                            ex_t[kc][:, z * 512:(z + 1) * 512],
                            start=(kc == 0), stop=(kc == KC - 1),
                        )

                for kc in range(KC):
                    sc = scps.tile([128, 1024], F32, name="sc", tag="sc")
                    for z in range(2):
                        nc.tensor.matmul(
                            sc[:, z * 512:(z + 1) * 512],
                            kT[t][base:base + D, kc * 128:(kc + 1) * 128],
                            qT[t][base:base + D, q0 + z * 512: q0 + (z + 1) * 512],
                            start=True, stop=True,
                        )
                    ex = expool.tile([128, 1024], BF16, name="ex", tag="ex")
                    if 8 <= gi < 12 and kc in (13, 15):
                        # Schraudolph fast exp on DVE: keeps ACT below the
                        # PE pace in these sparsely-filled groups.
                        ex32 = ex32pool.tile([128, 1024], mybir.dt.int32,
                                             name="ex32", tag="ex32")
                        nc.vector.scalar_tensor_tensor(
                            out=ex32[:], in0=sc[:],
                            scalar=12102203.16156148 * scale,
                            in1=bconst[:],
                            op0=mybir.AluOpType.mult, op1=mybir.AluOpType.add,
                        )
                        nc.vector.tensor_copy(out=ex[:], in_=ex32[:].bitcast(F32))
                    else:
                        nc.scalar.activation(
                            out=ex[:], in_=sc[:],
                            func=mybir.ActivationFunctionType.Exp, scale=scale,
                        )
                    ex_t[kc] = ex
                    for fn in fillers.pop((gi, kc), ()):
                        fn()
                    if kc >= AV_LAG:
                        emit_av(kc - AV_LAG)
                for kc in range(KC - AV_LAG, KC):
                    emit_av(kc)

                # Normalization: den row -> reciprocal -> broadcast -> mul.
                den_row = dpool.tile([1, 1024], F32, name="den_row", tag="den_row")
                bc = bcpool.tile([64, 1024], F32, name="bc", tag="bc")
                for z in range(2):
                    zs = slice(z * 512, (z + 1) * 512)
                    nc.vector.reciprocal(out=den_row[:, zs], in_=av[64:65, zs])
                    nc.sync.dma_start(out=rcp_dram[gi, zs], in_=den_row[:, zs])
                    row = rcp_dram[gi, zs]
                    nc.sync.dma_start(
                        out=bc[:, zs],
                        in_=bass.AP(tensor=row.tensor, offset=row.offset,
                                    ap=[[0, 64], [1, 512]]),
                    )
                for z in range(2):
                    zs = slice(z * 512, (z + 1) * 512)
                    nc.vector.tensor_mul(
                        out=oT[t][base:base + D, q0 + z * 512:q0 + (z + 1) * 512],
                        in0=av[0:64, zs], in1=bc[:, zs],
                    )

            assert not fillers, f"unemitted fillers: {list(fillers)}"

            # tail: final projection
            for gc in range(GC):
                emit_po3(gc)

    split_multi_waits(nc)
    return nc


_CACHE = {}


def make_in_maps(x, Wq, Wk, Wv, Wp):
    WqT = np.asarray(Wq).T.astype(NP_BF16)
    WkT = np.asarray(Wk).T.astype(NP_BF16)
    WvT = np.asarray(Wv).T.astype(NP_BF16)
    WpT = np.asarray(Wp).T.astype(NP_BF16)
    in_maps = []
    for core in range(N_CORES):
        b, s = core // 2, core % 2
        osl = slice(s * O_LOC, (s + 1) * O_LOC)
        in_maps.append({
            "xT": np.ascontiguousarray(x[b].T).astype(NP_BF16),
            "wqT": np.ascontiguousarray(WqT[:, osl]),
            "wkT": np.ascontiguousarray(WkT[:, osl]),
            "wvT": np.ascontiguousarray(WvT[:, osl]),
            "wpT": np.ascontiguousarray(WpT[osl, :]),
        })
    return in_maps


def kernel(x, Wq, Wk, Wv, Wp, bp):
    x = np.ascontiguousarray(np.asarray(x, dtype=np.float32))
    in_maps = make_in_maps(x, np.asarray(Wq), np.asarray(Wk), np.asarray(Wv),
                           np.asarray(Wp))
    if "nc" not in _CACHE:
        _CACHE["nc"] = build_program()
    res = run_bass_kernel_spmd(_CACHE["nc"], in_maps, list(range(N_CORES)))
    out = np.zeros((B, G, C), np.float32)
    bp = np.asarray(bp, dtype=np.float32)
    for b in range(B):
        r0, r1 = res.results[2 * b], res.results[2 * b + 1]
        out[b] = (r0["out_p"].astype(np.float32) + r0["out_q"].astype(np.float32)
                  + r1["out_p"].astype(np.float32) + r1["out_q"].astype(np.float32)
                  + bp)
    return out
